# revision 1
# baseline (speedup 1.0000x reference)
"""Trainium2 Bass kernel for nn_CapXLayer (CapsNet-style layer).

Sharding: data-parallel over batch. 8 batches -> 8 NeuronCores, one batch
per core. All parameters replicated. Full inputs in, full output out.

Per-core dataflow (one batch, CH-layout [channels, pixels], px chunks of 512):
  phase 1: relu(x) -> conv1 (1x1 grouped, block-diag f32r matmuls)
           -> conv2 (3x3 grouped SAME, 9 taps x 2 halves accumulated in PSUM,
              input kept in a zero-padded [128,66,66] layout)
  phase 2 (per 512-px chunk):
           conv3 (per-capsule 1x1, K=32) -> u_raw[ic] chunk tiles (b3 folded
           into every later read via the STT scalar slot)
           nsq = sum_od (u+b3)^2 via ACT Square + masked-ones PE reduce
           g_u = squash factor; dynamic routing (ITERS=3) algebraically
           folded to 5 elementwise U-passes (STT) + PE reduces/replications
  tail:    spatial capsule attention (stats packed [64,512] chunk-major)
           + residual + store
"""

import numpy as np

import concourse.bass as bass
import concourse.bacc as bacc
import concourse.tile as tile
import concourse.mybir as mybir
from concourse.bass_utils import run_bass_kernel_spmd

F32 = mybir.dt.float32
F32R = mybir.dt.float32r
BF16 = mybir.dt.bfloat16
AF = mybir.ActivationFunctionType
OP = mybir.AluOpType

IC, IND, MID, OC, OD = 8, 16, 32, 8, 16
B, H, W = 8, 64, 64
PX = H * W            # 4096
CS = 512              # pixels per chunk
NCH = PX // CS        # 8 chunks
RPC = CS // W         # rows per chunk = 8

TAPS = [(dy, dx) for dy in (-1, 0, 1) for dx in (-1, 0, 1)]


# ---------------------------------------------------------------- host prep
def _prep_consts(w1, b1, w2, b2, w3, b3, attn_w, attn_b):
    """Precompute matmul-ready weight layouts and constant matrices."""
    c = {}
    # conv1 lhsT: [128, 128]; rows 64h..64h+63 hold half h's lhsT so the
    # lhsT slice shares its base partition with the rhs x-slice
    w1L = np.zeros((128, 128), np.float32)
    for h in range(2):
        for g in range(4):
            gg = h * 4 + g
            w1L[64 * h + g * 16:64 * h + (g + 1) * 16,
                g * 32:(g + 1) * 32] = w1[gg * 32:(gg + 1) * 32, :, 0, 0].T
    c["w1L"] = w1L
    # q-layout: u/s partition p = 32*t + 4*oc + odw  (od = 4*t + odw).
    # perm[p] = channel index within an ic block = 16*oc + od.
    pq = np.arange(128)
    t_of = pq >> 5
    oc_of = (pq >> 2) & 7
    odw_of = pq & 3
    perm = 16 * oc_of + 4 * t_of + odw_of          # q-partition -> ch
    c_perm = perm
    # conv2 lhsT: [128, 2, 9, 128]
    w2L = np.zeros((128, 2, 9, 128), np.float32)
    for h in range(2):
        for t, (dy, dx) in enumerate(TAPS):
            for g in range(4):
                gg = h * 4 + g
                w2L[g * 32:(g + 1) * 32, h, t, g * 32:(g + 1) * 32] = \
                    w2[gg * 32:(gg + 1) * 32, :, dy + 1, dx + 1].T
    c["w2L"] = w2L
    # conv3 lhsT: [96, 8, 128].  ic's lhsT sits at rows 32g..32g+31 where
    # g = ic%4 for g<3 (partition-aligned with its h2 slice); the g==3
    # capsules are staged to base 0 (PE quadrant 3 is unusable), so their
    # lhsT sits at rows 0..31.
    w3L = np.zeros((96, 8, 128), np.float32)
    for ic in range(IC):
        g = ic % 4
        r0 = 32 * g if g < 3 else 0
        w3L[r0:r0 + 32, ic, :] = w3[ic * 128 + perm, :, 0, 0].T
    c["w3L"] = w3L
    # biases as per-partition columns
    c["b1s"] = np.stack([b1[0:128], b1[128:256]], axis=1).astype(np.float32)
    c["b2s"] = np.stack([b2[0:128], b2[128:256]], axis=1).astype(np.float32)
    c["b3s"] = b3.reshape(IC, 128)[:, perm].T.astype(np.float32).copy()
    # reduction / replication constants (all in q-layout row space)
    onesA = (oc_of[:, None] == np.arange(OC)[None, :]).astype(np.float32)
    c["onesA"] = onesA                                               # [128, 8]
    j = np.arange(64)
    c["onesB"] = (oc_of[:, None] == (j % OC)[None, :]).astype(np.float32)
    # redM[:, ic, :]: [128 q-rows, 64 (ic',oc')] masked ones16 reduce —
    # sums od into rows 8ic..8ic+7 only; 8 accumulating matmuls build the
    # whole [64, CS] tile with every tile_position at base 0.
    redM = np.zeros((128, 8, 64), np.float32)
    for ic in range(IC):
        redM[pq, ic, ic * 8 + oc_of] = 1.0
    c["redM"] = redM
    import ml_dtypes
    c["redM16"] = redM.astype(ml_dtypes.bfloat16)
    # u_p layout: tile th=(t,h) holds rows j=(odw,icg,oc) = 32*odw+8*icg+oc
    # with value u[ic=4h+icg, ch=16*oc+4*t+odw].  conv3 lhsT per th:
    w3P = np.zeros((128, 8, 128), np.float32)
    b3P = np.zeros((128, 8), np.float32)
    jj = np.arange(128)
    odw_j = jj >> 5
    icg_j = (jj >> 3) & 3
    oc_j = jj & 7
    for th in range(8):
        t, h = th >> 1, th & 1
        ch_full = (4 * h + icg_j) * 128 + 16 * oc_j + 4 * t + odw_j
        for j in range(128):
            k = icg_j[j] * 32 + np.arange(MID)
            w3P[k, th, j] = w3[ch_full[j], :, 0, 0]
        b3P[:, th] = b3[ch_full]
    c["w3P"] = w3P
    c["b3P"] = b3P
    # accum masks: product rows j=(odw,icg,oc) -> s_q rows 32t+4oc+odw,
    # summing icg.  accM4[:, t, :]; accMh4 folds the iter-1 factor 0.5.
    accM4 = np.zeros((128, 4, 128), np.float32)
    for t in range(4):
        accM4[jj, t, 32 * t + 4 * oc_j + odw_j] = 1.0
    c["accM4"] = accM4.astype(ml_dtypes.bfloat16)
    c["accMh4"] = (0.5 * accM4).astype(ml_dtypes.bfloat16)
    # repP[:, h, :]: [64 (ic,oc), 128 (odw,icg,oc)] — replicates the half-h
    # block of a [64, CS] c-tile into u_p broadcast row space
    repP = np.zeros((64, 2, 128), np.float32)
    for h in range(2):
        repP[8 * (4 * h + icg_j) + oc_j, h, jj] = 1.0
    c["repP"] = repP.astype(ml_dtypes.bfloat16)
    # attention tail constants (avg packed [64,512], partition = 8c+oc)
    sel64 = np.zeros((64, 8), np.float32)        # sum over chunk blocks
    rep64 = np.zeros((8, 64), np.float32)        # replicate [8,1] -> [64,1]
    for cc in range(NCH):
        for ocv in range(OC):
            sel64[cc * 8 + ocv, ocv] = 1.0
            rep64[ocv, cc * 8 + ocv] = 1.0
    c["sel64"] = sel64
    c["rep64"] = rep64
    # selrep[:, c, :]: [64, 8, 128] — replicate rows 8c..8c+7 (the chunk's
    # [8,CS] sigmoid block) over od into [128, CS]
    selrep = np.zeros((64, 8, 128), np.float32)
    for cc in range(NCH):
        selrep[cc * 8 + oc_of, cc, pq] = 1.0
    c["selrep"] = selrep
    c["zpad"] = np.zeros((128, 132), np.float32)
    c["aw"] = attn_w.reshape(1, OC).astype(np.float32).copy()
    c["ab"] = attn_b.reshape(1, OC).astype(np.float32).copy()
    return c


F32_CONSTS = {"b1s", "b2s", "b3s", "b3P", "aw", "ab", "sel64", "rep64"}
BF16_CONSTS = {"redM16", "accM4", "accMh4", "repP"}

CONST_SHAPES = {
    "w1L": [128, 128], "w2L": [128, 2, 9, 128], "w3L": [96, 8, 128],
    "w3P": [128, 8, 128],
    "b1s": [128, 2], "b2s": [128, 2], "b3s": [128, 8], "b3P": [128, 8],
    "onesA": [128, 8], "onesB": [128, 64],
    "redM": [128, 8, 64], "redM16": [128, 8, 64],
    "accM4": [128, 4, 128], "accMh4": [128, 4, 128], "repP": [64, 2, 128],
    "sel64": [64, 8], "rep64": [8, 64],
    "selrep": [64, 8, 128], "aw": [1, 8], "ab": [1, 8], "zpad": [128, 132],
}


# ---------------------------------------------------------------- kernel IR
def _g_chain(nc, pool, nsq_ps, cb_eps, cb_half):
    """g = nsq / ((0.5+nsq) * (sqrt(nsq+1e-6)+1e-6)) on a [64, CS] tile.

    nsq_ps may be in PSUM. Returns a rotating SBUF tile holding g."""
    shp = [64, CS]
    r_t = pool.tile(shp, F32, name="gch_r", tag="gch_r")
    nc.scalar.activation(out=r_t[:], in_=nsq_ps, func=AF.Sqrt,
                         bias=cb_eps[:64], scale=1.0)
    a_t = pool.tile(shp, F32, name="gch_a", tag="gch_a")
    nc.scalar.activation(out=a_t[:], in_=nsq_ps, func=AF.Identity,
                         bias=cb_half[:64], scale=1.0)
    den = pool.tile(shp, F32, name="gch_d", tag="gch_d")
    nc.vector.scalar_tensor_tensor(out=den[:], in0=r_t[:], scalar=1e-6,
                                   in1=a_t[:], op0=OP.add, op1=OP.mult)
    rg = pool.tile(shp, F32, name="gch_rg", tag="gch_rg")
    nc.vector.reciprocal_approx_fast(out=rg[:], in_=den[:])
    g_t = pool.tile(shp, BF16, name="gch_g", tag="gch_g")
    nc.vector.tensor_tensor(out=g_t[:], in0=nsq_ps, in1=rg[:], op=OP.mult)
    return g_t


def build_nc(num_devices=8, stage=99):
    nc = bacc.Bacc("TRN2", target_bir_lowering=False, debug=False,
                   num_devices=num_devices)

    io = {}
    io["x"] = nc.dram_tensor("x", [128, PX], F32R, kind="ExternalInput").ap()
    for name, shp in CONST_SHAPES.items():
        dt = (F32 if name in F32_CONSTS else
              BF16 if name in BF16_CONSTS else F32R)
        io[name] = nc.dram_tensor(name, shp, dt, kind="ExternalInput").ap()
    out_dram = nc.dram_tensor("out", [128, PX], F32, kind="ExternalOutput").ap()

    with tile.TileContext(nc) as tc:
        _body(tc, io, out_dram, stage)
    nc.compile()
    return nc


def _body(tc, io, out_dram, stage=99):
    nc = tc.nc

    import contextlib
    ctx = contextlib.ExitStack()
    with ctx:
        consts = ctx.enter_context(tc.tile_pool(name="consts", bufs=1))
        cs_t = {}
        for name, shp in CONST_SHAPES.items():
            dt = (F32 if name in F32_CONSTS else
                  BF16 if name in BF16_CONSTS else F32R)
            t = consts.tile(shp, dt, name=name, tag=name)
            nc.sync.dma_start(out=t[:], in_=io[name])
            cs_t[name] = t

        cb_eps = consts.tile([128, 1], F32, name="cb_eps", tag="cb_eps")
        nc.vector.memset(cb_eps[:], 1e-6)
        cb_half = consts.tile([128, 1], F32, name="cb_half", tag="cb_half")
        nc.vector.memset(cb_half[:], 0.5)

        persist = ctx.enter_context(tc.tile_pool(name="persist", bufs=1))
        sf_sb = persist.tile([128, PX], F32, name="sf", tag="sf")

        # banded conv pipeline state: h1b covers image rows 8c-1..8c+8 at
        # idx r = R-(8c-1); h2b is the chunk's conv2 output
        hb = ctx.enter_context(tc.tile_pool(name="hb", bufs=3))
        zp = cs_t["zpad"]

        def conv1_band(c, ppcv):
            r_lo = max(8 * c - 1, 0)
            r_hi = min(8 * c + 9, H)
            n = r_hi - r_lo
            xb = hb.tile([128, 10 * W], F32R, name="xb", tag="xb")
            nc.sync.dma_start(out=xb[:, 0:n * W],
                              in_=io["x"][:, r_lo * W:r_hi * W])
            rxb = hb.tile([128, 10 * W], F32R, name="rxb", tag="rxb")
            nc.scalar.activation(out=rxb[:, 0:n * W], in_=xb[:, 0:n * W],
                                 func=AF.Relu)
            h1b = [hb.tile([128, 10, 66], F32R, name=f"h1b{h}",
                           tag=f"h1b{h}") for h in range(2)]
            for h in range(2):
                # zero padding columns (and edge rows at image boundary)
                nc.sync.dma_start(
                    out=h1b[h][:, :, 0:1],
                    in_=zp[:, 0:10].rearrange("p (a b) -> p a b", b=1))
                nc.sync.dma_start(
                    out=h1b[h][:, :, 65:66],
                    in_=zp[:, 0:10].rearrange("p (a b) -> p a b", b=1))
                if c == 0:
                    nc.sync.dma_start(out=h1b[h][:, 0, :], in_=zp[:, 0:66])
                if c == NCH - 1:
                    nc.sync.dma_start(out=h1b[h][:, 9, :], in_=zp[:, 0:66])
                idx_lo = r_lo - (8 * c - 1)
                k1 = n // 2
                for part, (ro, k) in enumerate(((0, k1), (k1, n - k1))):
                    ps = ppcv.tile([128, CS], F32, name="cvps", tag="c3ps")
                    nc.tensor.matmul(
                        ps[:, 0:k * W],
                        cs_t["w1L"][h * 64:(h + 1) * 64, :],
                        rxb[h * 64:(h + 1) * 64, ro * W:(ro + k) * W],
                        start=True, stop=True)
                    nc.scalar.activation(
                        out=h1b[h][:, idx_lo + ro:idx_lo + ro + k, 1:65],
                        in_=ps[:, 0:k * W].rearrange("p (a b) -> p a b", a=k),
                        func=AF.Relu, bias=cs_t["b1s"][:, h:h + 1], scale=1.0)
            return h1b

        def conv2_band(c, h1b, ppcv):
            h2b = [hb.tile([128, CS], F32R, name=f"h2b{h}", tag=f"h2b{h}")
                   for h in range(2)]
            for h in range(2):
                ps = ppcv.tile([128, CS], F32, name="cvps", tag="c3ps")
                for t, (dy, dx) in enumerate(TAPS):
                    nc.tensor.matmul(
                        ps[:],
                        cs_t["w2L"][:, h, t, :],
                        h1b[h][:, 1 + dy:9 + dy, 1 + dx:65 + dx],
                        start=(t == 0), stop=(t == len(TAPS) - 1))
                nc.scalar.activation(
                    out=h2b[h][:], in_=ps[:],
                    func=AF.Relu, bias=cs_t["b2s"][:, h:h + 1], scale=1.0)
            return h2b

        # ---------------- phase 2: conv3 + routing, per chunk ------------
        scr = ctx.enter_context(tc.tile_pool(name="scr", bufs=4))
        sm = ctx.enter_context(tc.tile_pool(name="sm", bufs=2))
        up = ctx.enter_context(tc.tile_pool(name="up", bufs=2))
        upp = ctx.enter_context(tc.tile_pool(name="upp", bufs=2))
        cbp = ctx.enter_context(tc.tile_pool(name="cbp", bufs=2))
        ph2ps = contextlib.ExitStack()
        ppc3 = ph2ps.enter_context(
            tc.tile_pool(name="ppc3", bufs=3, space="PSUM"))
        ppred = ph2ps.enter_context(
            tc.tile_pool(name="ppred", bufs=2, space="PSUM"))
        pps = ph2ps.enter_context(
            tc.tile_pool(name="pps", bufs=2, space="PSUM"))

        b3s = cs_t["b3s"]
        avg64 = persist.tile([64, CS], F32, name="avg64", tag="avg64")

        def accum_pass(u_pt, cT, masks, s_ps):
            """s_ps[32t+4oc+odw] = sum_ic c[ic,oc]*u_p[(t,h)][odw,icg,oc].

            cT is a [64, CS] bf16 SBUF tile in (ic,oc) row space; its two
            32-row halves are partition-broadcast over odw via DMA so every
            DVE multiply runs in 2x bf16 mode off one shared tile."""
            cb = []
            for h in range(2):
                rep_ps = ppc3.tile([128, CS], F32, name="c3ps", tag="c3ps")
                nc.tensor.matmul(rep_ps[:], cs_t["repP"][:, h, :], cT[:],
                                 start=True, stop=True)
                cbt = cbp.tile([128, CS], BF16, name=f"cb{h}", tag=f"cb{h}")
                nc.scalar.copy(out=cbt[:], in_=rep_ps[:])
                cb.append(cbt)
            for th in range(8):
                t, h = th >> 1, th & 1
                p_t = scr.tile([128, CS], BF16, name="pp", tag="pp")
                eng = nc.gpsimd if th >= 6 else nc.vector
                eng.tensor_tensor(out=p_t[:], in0=u_pt[th][:],
                                  in1=cb[h][:], op=OP.mult)
                nc.tensor.matmul(s_ps[:], masks[:, t, :], p_t[:],
                                 start=(th == 0), stop=(th == 7))

        def stt_prods_reduce(u_c, in1_ps, red_ps):
            """red_ps rows 8ic..8ic+7 = sum_od u_b[ic] * in1 — all-bf16
            operands so the DVE multiplies run in 2x mode."""
            s16 = scr.tile([128, CS], BF16, name="s16", tag="s16")
            nc.scalar.copy(out=s16[:], in_=in1_ps)
            for ic in range(IC):
                q_t = scr.tile([128, CS], BF16, name="q16", tag="q16")
                eng = nc.gpsimd if ic >= 6 else nc.vector
                eng.tensor_tensor(out=q_t[:], in0=u_c[ic][:],
                                  in1=s16[:], op=OP.mult)
                nc.tensor.matmul(red_ps[:],
                                 cs_t["redM16"][:, ic, :],
                                 q_t[:],
                                 start=(ic == 0), stop=(ic == IC - 1))

        h1b_cur = conv1_band(0, ppc3)
        for c in range(NCH):
            csl = slice(c * CS, (c + 1) * CS)

            h2b = conv2_band(c, h1b_cur, ppc3)
            if c + 1 < NCH:
                h1b_cur = conv1_band(c + 1, ppc3)

            # conv3 + square/nsq + u chunk tiles
            u_c = []
            nsq_ps = ppred.tile([64, CS], F32, name="red64", tag="red64")
            for ic in range(IC):
                g = ic % 4
                if g < 3:
                    lhsT = cs_t["w3L"][32 * g:32 * g + 32, ic, :]
                    rhs = h2b[ic // 4][32 * g:32 * g + 32, :]
                else:
                    h2st = scr.tile([32, CS], F32R, name="h2s", tag="h2s")
                    nc.scalar.copy(out=h2st[:],
                                   in_=h2b[ic // 4][96:128, :])
                    lhsT = cs_t["w3L"][0:32, ic, :]
                    rhs = h2st[:]
                ups = ppc3.tile([128, CS], F32, name="c3ps", tag="c3ps")
                nc.tensor.matmul(ups[:], lhsT,
                                 rhs, start=True, stop=True)
                u_t = up.tile([128, CS], BF16, name=f"u{ic}", tag=f"u{ic}")
                nc.scalar.activation(out=u_t[:], in_=ups[:], func=AF.Identity,
                                     bias=b3s[:, ic:ic + 1], scale=1.0)
                u_c.append(u_t)
                # nsq accumulates (u+b3)^2 via a DVE bf16 square (2x mode)
                eng = nc.vector if ic % 2 == 0 else nc.gpsimd
                sq_t = scr.tile([128, CS], BF16, name="sq", tag="sq")
                eng.tensor_tensor(out=sq_t[:], in0=u_t[:], in1=u_t[:],
                                  op=OP.mult)
                nc.tensor.matmul(nsq_ps[:],
                                 cs_t["redM16"][:, ic, :],
                                 sq_t[:],
                                 start=(ic == 0), stop=(ic == IC - 1))

            # u_p layout tiles for the three s-accumulation passes
            u_pt = []
            for th in range(8):
                upsp = ppc3.tile([128, CS], F32, name="c3ps", tag="c3ps")
                nc.tensor.matmul(upsp[:], cs_t["w3P"][:, th, :],
                                 h2b[th & 1][:], start=True, stop=True)
                u_t = upp.tile([128, CS], BF16, name=f"up{th}", tag=f"up{th}")
                nc.scalar.activation(out=u_t[:], in_=upsp[:], func=AF.Identity,
                                     bias=cs_t["b3P"][:, th:th + 1], scale=1.0)
                u_pt.append(u_t)

            g_u = _g_chain(nc, sm, nsq_ps[:], cb_eps, cb_half)
            if stage <= 2:
                nc.vector.tensor_copy(out=sf_sb[0:64, csl], in_=g_u[:])
                continue

            # iter 1: s1 = sum_ic 0.5*g_u*(u+b3)  (0.5 folded into accMh4)
            s1_ps = pps.tile([128, CS], F32, name="sacc", tag="sacc")
            accum_pass(u_pt, g_u, cs_t["accMh4"], s1_ps)

            # squash(s1) factor (ic-replicated via onesB)
            s1sq = scr.tile([128, CS], F32R, name="sq", tag="sq")
            nc.scalar.activation(out=s1sq[:], in_=s1_ps[:], func=AF.Square)
            ns1_ps = ppred.tile([64, CS], F32, name="red64", tag="red64")
            nc.tensor.matmul(ns1_ps[:], cs_t["onesB"][:],
                             s1sq[:], start=True, stop=True)
            g1 = _g_chain(nc, sm, ns1_ps[:], cb_eps, cb_half)

            # d1 ; b2 = g_u*g1*d1 ; c2 ; ct2 = c2*g_u
            d1_ps = ppred.tile([64, CS], F32, name="red64", tag="red64")
            stt_prods_reduce(u_c, s1_ps[:], d1_ps)
            gg1 = sm.tile([64, CS], F32, name="gg", tag="gg")
            nc.vector.tensor_tensor(out=gg1[:], in0=g1[:], in1=g_u[:],
                                    op=OP.mult)
            b2 = sm.tile([64, CS], F32, name="b2", tag="b2")
            nc.vector.tensor_tensor(out=b2[:], in0=d1_ps[:], in1=gg1[:],
                                    op=OP.mult)
            c2 = sm.tile([64, CS], BF16, name="c2", tag="c2")
            nc.scalar.activation(out=c2[:], in_=b2[:], func=AF.Sigmoid)
            if stage <= 3:
                nc.vector.tensor_copy(out=sf_sb[0:64, csl], in_=c2[:])
                continue
            ct2 = sm.tile([64, CS], BF16, name="ct2", tag="ct2")
            nc.vector.tensor_tensor(out=ct2[:], in0=c2[:], in1=g_u[:],
                                    op=OP.mult)

            # iter 2
            s2_ps = pps.tile([128, CS], F32, name="sacc", tag="sacc")
            accum_pass(u_pt, ct2, cs_t["accM4"], s2_ps)

            s2sq = scr.tile([128, CS], F32R, name="sq", tag="sq")
            nc.scalar.activation(out=s2sq[:], in_=s2_ps[:], func=AF.Square)
            ns2_ps = ppred.tile([64, CS], F32, name="red64", tag="red64")
            nc.tensor.matmul(ns2_ps[:], cs_t["onesB"][:],
                             s2sq[:], start=True, stop=True)
            g2 = _g_chain(nc, sm, ns2_ps[:], cb_eps, cb_half)

            d2_ps = ppred.tile([64, CS], F32, name="red64", tag="red64")
            stt_prods_reduce(u_c, s2_ps[:], d2_ps)
            gg2 = sm.tile([64, CS], F32, name="gg", tag="gg")
            nc.vector.tensor_tensor(out=gg2[:], in0=g2[:], in1=g_u[:],
                                    op=OP.mult)
            b3r = sm.tile([64, CS], F32, name="b2", tag="b2")
            nc.vector.tensor_tensor(out=b3r[:], in0=d2_ps[:], in1=gg2[:],
                                    op=OP.mult)
            nc.vector.tensor_tensor(out=b3r[:], in0=b3r[:], in1=b2[:],
                                    op=OP.add)
            c3 = sm.tile([64, CS], BF16, name="c2", tag="c2")
            nc.scalar.activation(out=c3[:], in_=b3r[:], func=AF.Sigmoid)

            # final mix: sf = sum_ic c3*(u+b3)
            sf_ps = pps.tile([128, CS], F32, name="sacc", tag="sacc")
            accum_pass(u_pt, c3, cs_t["accM4"], sf_ps)
            nc.scalar.copy(out=sf_sb[:, csl], in_=sf_ps[:])

        if stage <= 4:
            ph2ps.close()
            nc.sync.dma_start(out=out_dram, in_=sf_sb[:])
            return

        # ---------------- tail: spatial capsule attention ----------------
        ph2ps.close()
        tt = ctx.enter_context(tc.tile_pool(name="tt", bufs=1))
        dramp = ctx.enter_context(tc.tile_pool(name="dramp", bufs=1,
                                               space="DRAM"))
        ppt = ctx.enter_context(tc.tile_pool(name="ppt", bufs=1, space="PSUM"))

        mh = tt.tile([128, 1], F32, name="mh", tag="mh")
        nc.vector.reduce_sum(out=mh[:], in_=sf_sb[:], axis=mybir.AxisListType.X)
        nc.scalar.mul(mh[:], mh[:], 1.0 / PX)

        # avg packed [64, CS]: partition 8c+oc holds chunk c's avg row oc
        for c in range(NCH):
            csl = slice(c * CS, (c + 1) * CS)
            scrc = scr.tile([128, CS], F32R, name="p", tag="p")
            nc.vector.tensor_scalar(out=scrc[:], in0=sf_sb[:, csl],
                                    scalar1=mh[:], scalar2=None, op0=OP.mult)
            av_ps = ppt.tile([8, CS], F32, name="avgc", tag="avgc")
            nc.tensor.matmul(av_ps[:], cs_t["onesA"][:],
                             scrc[:], start=True, stop=True)
            # compute engines need 32-aligned start partitions; bounce via
            # SBUF and let DMA scatter to partition 8c
            avst = scr.tile([8, CS], F32, name="avst", tag="h2s")
            nc.scalar.copy(out=avst[:], in_=av_ps[:])
            nc.sync.dma_start(out=avg64[8 * c:8 * c + 8, :], in_=avst[:])
        if stage <= 41:
            nc.sync.dma_start(out=out_dram, in_=sf_sb[:])
            return

        rowsum = tt.tile([64, 1], F32, name="rowsum", tag="rowsum")
        nc.vector.reduce_sum(out=rowsum[:], in_=avg64[:],
                             axis=mybir.AxisListType.X)
        # gather the 64 per-(chunk,oc) row sums onto one partition, reduce
        # the chunk axis there, and broadcast back — avoids tiny PE matmuls
        rowsT = tt.tile([1, 64], F32, name="rowsT", tag="rowsT")
        nc.sync.dma_start(out=rowsT[:], in_=rowsum[:])
        m_row = tt.tile([1, 8], F32, name="m_row", tag="m_row")
        nc.vector.reduce_sum(
            out=m_row[:],
            in_=bass.AP(tensor=rowsT.tensor, offset=rowsT.offset,
                        ap=[[64, 1], [1, 8], [8, 8]]),
            axis=mybir.AxisListType.X)
        nc.scalar.mul(m_row[:], m_row[:], 1.0 / PX)
        mrow_d = dramp.tile([1, 8], F32, name="mrow_d", tag="mrow_d")
        nc.sync.dma_start(out=mrow_d[:], in_=m_row[:])
        m64 = tt.tile([64, 1], F32, name="m64", tag="m64")
        nc.sync.dma_start(
            out=m64[:],
            in_=bass.AP(tensor=mrow_d.tensor, offset=mrow_d.offset,
                        ap=[[0, 8], [1, 8]]))
        cen = tt.tile([64, CS], F32, name="cen", tag="cen")
        nc.vector.tensor_scalar(out=cen[:], in0=avg64[:], scalar1=m64[:],
                                scalar2=None, op0=OP.subtract)
        vjunk = tt.tile([64, CS], F32, name="vjunk", tag="vjunk")
        nc.vector.tensor_tensor(out=vjunk[:], in0=cen[:], in1=cen[:],
                                op=OP.mult)
        v64 = tt.tile([64, 1], F32, name="v64", tag="v64")
        nc.vector.reduce_sum(out=v64[:], in_=vjunk[:],
                             axis=mybir.AxisListType.X)
        vT = tt.tile([1, 64], F32, name="vT", tag="vT")
        nc.sync.dma_start(out=vT[:], in_=v64[:])
        var8 = tt.tile([1, 8], F32, name="var8", tag="var8")
        nc.vector.reduce_sum(
            out=var8[:],
            in_=bass.AP(tensor=vT.tensor, offset=vT.offset,
                        ap=[[64, 1], [1, 8], [8, 8]]),
            axis=mybir.AxisListType.X)
        sd8 = tt.tile([1, 8], F32, name="sd8", tag="sd8")
        nc.scalar.activation(out=sd8[:], in_=var8[:], func=AF.Sqrt,
                             bias=0.0, scale=1.0 / (PX - 1))
        nc.scalar.activation(out=sd8[:], in_=sd8[:], func=AF.Identity,
                             bias=cb_eps[:1], scale=1.0)
        rsd8 = tt.tile([1, 8], F32, name="rsd8", tag="rsd8")
        nc.vector.reciprocal(out=rsd8[:], in_=sd8[:])
        rsdw8 = tt.tile([1, 8], F32, name="rsdw8", tag="rsdw8")
        nc.vector.tensor_tensor(out=rsdw8[:], in0=rsd8[:], in1=cs_t["aw"][:],
                                op=OP.mult)
        if stage <= 42:
            nc.sync.dma_start(out=out_dram, in_=sf_sb[:])
            return
        rsdw_d = dramp.tile([1, 8], F32, name="rsdw_d", tag="rsdw_d")
        nc.sync.dma_start(out=rsdw_d[:], in_=rsdw8[:])
        rw64 = tt.tile([64, 1], F32, name="rw64", tag="rw64")
        nc.sync.dma_start(
            out=rw64[:],
            in_=bass.AP(tensor=rsdw_d.tensor, offset=rsdw_d.offset,
                        ap=[[0, 8], [1, 8]]))
        ab64 = tt.tile([64, 1], F32, name="ab64", tag="ab64")
        nc.sync.dma_start(
            out=ab64[:],
            in_=bass.AP(tensor=io["ab"].tensor, offset=io["ab"].offset,
                        ap=[[0, 8], [1, 8]]))
        t2 = tt.tile([64, CS], F32, name="t2", tag="t2")
        nc.vector.tensor_scalar(out=t2[:], in0=cen[:], scalar1=rw64[:],
                                scalar2=ab64[:], op0=OP.mult, op1=OP.add)
        sig = tt.tile([64, CS], F32R, name="sig", tag="sig")
        nc.scalar.activation(out=sig[:], in_=t2[:], func=AF.Sigmoid)
        if stage <= 43:
            nc.sync.dma_start(out=out_dram, in_=sf_sb[:])
            return

        for c in range(NCH):
            csl = slice(c * CS, (c + 1) * CS)
            srep = ppt.tile([128, CS], F32, name="srep", tag="srep", bufs=2)
            nc.tensor.matmul(srep[:], cs_t["selrep"][:, c, :],
                             sig[:], start=True, stop=True)
            o1 = scr.tile([128, CS], F32, name="o1", tag="p")
            nc.vector.tensor_tensor(out=o1[:], in0=sf_sb[:, csl], in1=srep[:],
                                    op=OP.mult)
            xr = scr.tile([128, CS], F32R, name="sq", tag="sq")
            for t in range(4):
                qap = [[16 * PX, 8], [PX, 4], [1, CS]]
                nc.sync.dma_start(
                    out=xr[32 * t:32 * t + 32, :],
                    in_=bass.AP(tensor=io["x"].tensor,
                                offset=c * CS + 4 * t * PX, ap=qap))
            o2 = scr.tile([128, CS], F32, name="o2", tag="o2")
            nc.vector.tensor_tensor(out=o2[:], in0=o1[:], in1=xr[:], op=OP.add)
            for t in range(4):
                qap = [[16 * PX, 8], [PX, 4], [1, CS]]
                nc.sync.dma_start(
                    out=bass.AP(tensor=out_dram.tensor,
                                offset=c * CS + 4 * t * PX, ap=qap),
                    in_=o2[32 * t:32 * t + 32, :])


# ---------------------------------------------------------------- dispatch
_NC_CACHE = {}


def _get_nc():
    if "nc" not in _NC_CACHE:
        _NC_CACHE["nc"] = build_nc()
    return _NC_CACHE["nc"]


def kernel(x, w1, b1, w2, b2, w3, b3, attn_w, attn_b):
    x = np.ascontiguousarray(np.asarray(x, dtype=np.float32))
    consts = _prep_consts(
        np.asarray(w1, np.float32), np.asarray(b1, np.float32),
        np.asarray(w2, np.float32), np.asarray(b2, np.float32),
        np.asarray(w3, np.float32), np.asarray(b3, np.float32),
        np.asarray(attn_w, np.float32), np.asarray(attn_b, np.float32))
    consts = {k: np.ascontiguousarray(v) for k, v in consts.items()}

    nc = _get_nc()
    in_maps = []
    for b in range(B):
        m = {"x": x[b].reshape(128, PX).copy()}
        m.update(consts)
        in_maps.append(m)
    res = run_bass_kernel_spmd(nc, in_maps, core_ids=list(range(B)))
    out = np.zeros((B, 128, H, W), np.float32)
    for b in range(B):
        out[b] = res.results[b]["out"].reshape(128, H, W)
    return out



# revision 20
# speedup vs baseline: 1.5006x; 1.5006x over previous
"""Trainium2 Bass kernel for nn_CapXLayer (CapsNet-style layer).

Sharding: data-parallel over batch. 8 batches -> 8 NeuronCores, one batch
per core. All parameters replicated. Full inputs in, full output out.

Per-core dataflow (one batch, CH-layout [channels, pixels], px chunks of 512,
processed stage-major in groups of 4 chunks so every engine queue always has
independent cross-chunk work):

  conv:    relu(x) once -> conv1 (1x1 grouped) -> conv2 (3x3 grouped SAME,
           9 taps x 2 halves accumulated in PSUM, zero-padded [128,10,66]
           bands) -> u_pt[th] tiles (j-layout, bias folded at PSUM->SBUF copy)
  layouts: q = 16*oc + od            (s tiles; ALSO the output channel order,
                                      so the tail needs no permutation)
           j = 32*(oc>>1) + 16*(oc&1) + 4*icg + odw   (u_pt rows, th=(t,h),
                                      ic = 4h+icg, od = 4t+odw)
           r = 16*oc + ic            ("scattered" rows for per-(ic,oc) values:
                                      nsq/g/c/b/d; rows 16oc+8.. are unused
                                      garbage kept finite by zero mask columns)
           all replications (c -> cb[h], s16 -> drep[t]) are quadrant-local
           stream_shuffles under these layouts.
  routing: squash factors g = n/((0.5+n)*sqrt(n+1e-6)) computed with DVE
           pow/divide (no ACT Sqrt => the sigmoid act table stays resident,
           a single table load for the whole kernel)
  tail:    spatial capsule attention in natural layout + residual + store
"""

import numpy as np

import concourse.bass as bass
import concourse.bacc as bacc
import concourse.tile as tile
import concourse.mybir as mybir
from concourse.bass_utils import run_bass_kernel_spmd

F32 = mybir.dt.float32
F32R = mybir.dt.float32r
BF16 = mybir.dt.bfloat16
AF = mybir.ActivationFunctionType
OP = mybir.AluOpType

IC, IND, MID, OC, OD = 8, 16, 32, 8, 16
B, H, W = 8, 64, 64
PX = H * W            # 4096
CS = 512              # pixels per chunk
NCH = PX // CS        # 8 chunks
G = 4                 # chunks in flight (stage-major group)

TAPS = [(dy, dx) for dy in (-1, 0, 1) for dx in (-1, 0, 1)]

# j-layout helpers (u_pt rows): j = 32*(oc>>1) + 16*(oc&1) + 4*icg + odw
_j = np.arange(128)
J_OC = 2 * (_j >> 5) + ((_j >> 4) & 1)
J_ICG = (_j >> 2) & 3
J_ODW = _j & 3
# q-layout (s rows): q = 16*oc + od
_q = np.arange(128)
Q_OC = _q >> 4
Q_OD = _q & 15

# stream_shuffle masks (32-entry, per-quadrant; quadrant = oc>>1 everywhere)
_i = np.arange(32)
_i_oc1 = _i >> 4
_i_icg = (_i >> 2) & 3
_i_odw = _i & 3
MASK_CB = [list(16 * _i_oc1 + 4 * h + _i_icg) for h in range(2)]
MASK_DREP = [list(16 * _i_oc1 + 4 * t + _i_odw) for t in range(4)]


# ---------------------------------------------------------------- host prep
def _prep_consts(w1, b1, w2, b2, w3, b3, attn_w, attn_b):
    """Precompute matmul-ready weight layouts and constant matrices."""
    import ml_dtypes
    c = {}
    # conv1 lhsT: [128, 128]; rows 64h..64h+63 hold half h's lhsT so the
    # lhsT slice shares its base partition with the rhs x-slice
    w1L = np.zeros((128, 128), np.float32)
    for h in range(2):
        for g in range(4):
            gg = h * 4 + g
            w1L[64 * h + g * 16:64 * h + (g + 1) * 16,
                g * 32:(g + 1) * 32] = w1[gg * 32:(gg + 1) * 32, :, 0, 0].T
    c["w1L"] = w1L
    # conv2 lhsT: [128, 2, 9, 128]
    w2L = np.zeros((128, 2, 9, 128), np.float32)
    for h in range(2):
        for t, (dy, dx) in enumerate(TAPS):
            for g in range(4):
                gg = h * 4 + g
                w2L[g * 32:(g + 1) * 32, h, t, g * 32:(g + 1) * 32] = \
                    w2[gg * 32:(gg + 1) * 32, :, dy + 1, dx + 1].T
    c["w2L"] = w2L
    # biases as per-partition columns
    c["b1s"] = np.stack([b1[0:128], b1[128:256]], axis=1).astype(np.float32)
    c["b2s"] = np.stack([b2[0:128], b2[128:256]], axis=1).astype(np.float32)
    # conv3 lhsT (u_pt production): w3P[k, th, j], k = 32*icg + mid
    w3P = np.zeros((128, 8, 128), np.float32)
    b3P = np.zeros((128, 8), np.float32)
    for th in range(8):
        t, h = th >> 1, th & 1
        ch_full = (4 * h + J_ICG) * 128 + 16 * J_OC + 4 * t + J_ODW
        for j in range(128):
            k = J_ICG[j] * 32 + np.arange(MID)
            w3P[k, th, j] = w3[ch_full[j], :, 0, 0]
        b3P[:, th] = b3[ch_full]
    c["w3P"] = w3P
    c["b3P"] = b3P
    # accum masks: product rows j -> s rows q = 16*oc + 4t + odw, summing
    # (icg, h) via the 8-matmul PSUM accumulation. accMh folds iter-1's 0.5.
    accM = np.zeros((128, 4, 128), np.float32)
    for t in range(4):
        accM[_j, t, 16 * J_OC + 4 * t + J_ODW] = 1.0
    c["accM"] = accM.astype(ml_dtypes.bfloat16)
    c["accMh"] = (0.5 * accM).astype(ml_dtypes.bfloat16)
    # reduce masks: product rows j -> scattered rows r = 16*oc + ic,
    # summing (t, odw) via th accumulation
    redD = np.zeros((128, 8, 128), np.float32)
    for th in range(8):
        h = th & 1
        redD[_j, th, 16 * J_OC + 4 * h + J_ICG] = 1.0
    c["redD"] = redD.astype(ml_dtypes.bfloat16)
    # ns reduce: s rows q -> scattered rows 16*oc + ic, replicated over ic
    onesB = np.zeros((128, 128), np.float32)
    for ic in range(IC):
        onesB[_q, 16 * Q_OC + ic] = 1.0
    c["onesB"] = onesB.astype(ml_dtypes.bfloat16)
    # attention tail constants (avg packed [64,512], partition = 8c+oc)
    onesA = np.zeros((128, 8), np.float32)
    onesA[_q, Q_OC] = 1.0
    c["onesA"] = onesA
    sel64 = np.zeros((64, 8), np.float32)        # sum over chunk blocks
    rep64 = np.zeros((8, 64), np.float32)        # replicate [8,1] -> [64,1]
    for cc in range(NCH):
        for ocv in range(OC):
            sel64[cc * 8 + ocv, ocv] = 1.0
            rep64[ocv, cc * 8 + ocv] = 1.0
    c["sel64"] = sel64
    c["rep64"] = rep64
    # selrep[:, c, :]: [64, 8, 128] -- replicate rows 8c..8c+7 (the chunk's
    # [8,CS] sigmoid block) over od into q rows
    selrep = np.zeros((64, 8, 128), np.float32)
    for cc in range(NCH):
        selrep[cc * 8 + Q_OC, cc, _q] = 1.0
    c["selrep"] = selrep.astype(ml_dtypes.bfloat16)
    c["aw"] = attn_w.reshape(1, OC).astype(np.float32).copy()
    c["ab"] = attn_b.reshape(1, OC).astype(np.float32).copy()
    c["zpad"] = np.zeros((128, 66), np.float32)
    return c


F32_CONSTS = {"b1s", "b2s", "b3P", "aw", "ab", "sel64", "rep64"}
BF16_CONSTS = {"accM", "accMh", "redD", "onesB", "selrep"}

CONST_SHAPES = {
    "w1L": [128, 128], "w2L": [128, 2, 9, 128], "w3P": [128, 8, 128],
    "b1s": [128, 2], "b2s": [128, 2], "b3P": [128, 8],
    "accM": [128, 4, 128], "accMh": [128, 4, 128],
    "redD": [128, 8, 128], "onesB": [128, 128],
    "onesA": [128, 8], "sel64": [64, 8], "rep64": [8, 64],
    "selrep": [64, 8, 128], "aw": [1, 8], "ab": [1, 8], "zpad": [128, 66],
}


def build_nc(num_devices=8, stage=99):
    nc = bacc.Bacc("TRN2", target_bir_lowering=False, debug=False,
                   num_devices=num_devices)

    io = {}
    io["x"] = nc.dram_tensor("x", [128, PX], F32R, kind="ExternalInput").ap()
    for name, shp in CONST_SHAPES.items():
        dt = (F32 if name in F32_CONSTS else
              BF16 if name in BF16_CONSTS else F32R)
        io[name] = nc.dram_tensor(name, shp, dt, kind="ExternalInput").ap()
    out_dram = nc.dram_tensor("out", [128, PX], F32, kind="ExternalOutput").ap()

    with tile.TileContext(nc) as tc:
        _body(tc, io, out_dram, stage)
    nc.compile()
    return nc


def _body(tc, io, out_dram, stage=99):
    nc = tc.nc

    import contextlib
    ctx = contextlib.ExitStack()
    with ctx:
        consts = ctx.enter_context(tc.tile_pool(name="consts", bufs=1))
        cs_t = {}
        for name, shp in CONST_SHAPES.items():
            dt = (F32 if name in F32_CONSTS else
                  BF16 if name in BF16_CONSTS else F32R)
            t = consts.tile(shp, dt, name=name, tag=name)
            nc.sync.dma_start(out=t[:], in_=io[name])
            cs_t[name] = t

        persist = ctx.enter_context(tc.tile_pool(name="persist", bufs=1))
        x_sb = persist.tile([128, PX], F32R, name="x_sb", tag="x_sb")
        nc.sync.dma_start(out=x_sb[:], in_=io["x"])
        sf_sb = persist.tile([128, PX], F32, name="sf", tag="sf")
        avg64 = persist.tile([64, CS], F32, name="avg64", tag="avg64")
        zp = cs_t["zpad"]
        cb_eps = persist.tile([128, 1], F32, name="cb_eps", tag="cb_eps")
        nc.vector.memset(cb_eps[:], 1e-6)

        # ------------------------------------------------ pools
        ph2ps = contextlib.ExitStack()
        hb = ph2ps.enter_context(tc.tile_pool(name="hb", bufs=2))
        h2p = ph2ps.enter_context(tc.tile_pool(name="h2p", bufs=G))
        upp = ph2ps.enter_context(tc.tile_pool(name="upp", bufs=1))
        sm = ph2ps.enter_context(tc.tile_pool(name="sm", bufs=1))
        smt = ph2ps.enter_context(tc.tile_pool(name="smt", bufs=2))
        scr = ph2ps.enter_context(tc.tile_pool(name="scr", bufs=2))
        pcv = ph2ps.enter_context(
            tc.tile_pool(name="pcv", bufs=3, space="PSUM"))
        pred = ph2ps.enter_context(
            tc.tile_pool(name="pred", bufs=2, space="PSUM"))
        psa = ph2ps.enter_context(
            tc.tile_pool(name="psa", bufs=2, space="PSUM"))

        # TT engine round-robin: most to DVE, every third-ish to Pool
        rr = {"i": 0}

        def tt_eng():
            rr["i"] += 1
            return nc.gpsimd if rr["i"] % 4 == 0 else nc.vector

        # ------------------------------------------------ conv stages
        def conv1_band(c):
            r_lo = max(8 * c - 1, 0)
            r_hi = min(8 * c + 9, H)
            n = r_hi - r_lo
            idx_lo = r_lo - (8 * c - 1)
            rxb = hb.tile([128, 10 * W], F32R, name="rxb", tag="rxb")
            nc.scalar.activation(out=rxb[:, 0:n * W],
                                 in_=x_sb[:, r_lo * W:r_hi * W], func=AF.Relu)
            h1b = [hb.tile([128, 10, 66], F32R, name=f"h1b{h}",
                           tag=f"h1b{h}") for h in range(2)]
            for h in range(2):
                # zero the padding columns (and edge rows at image boundary)
                nc.sync.dma_start(
                    out=h1b[h][:, :, 0:1],
                    in_=zp[:, 0:10].rearrange("p (a b) -> p a b", b=1))
                nc.sync.dma_start(
                    out=h1b[h][:, :, 65:66],
                    in_=zp[:, 0:10].rearrange("p (a b) -> p a b", b=1))
                if c == 0:
                    nc.sync.dma_start(out=h1b[h][:, 0, :], in_=zp[:, 0:66])
                if c == NCH - 1:
                    nc.sync.dma_start(out=h1b[h][:, 9, :], in_=zp[:, 0:66])
                k1 = n // 2
                for ro, k in ((0, k1), (k1, n - k1)):
                    ps = pcv.tile([128, CS], F32, name="cvps", tag="cvps")
                    nc.tensor.matmul(
                        ps[:, 0:k * W],
                        cs_t["w1L"][h * 64:(h + 1) * 64, :],
                        rxb[h * 64:(h + 1) * 64, ro * W:(ro + k) * W],
                        start=True, stop=True)
                    nc.scalar.activation(
                        out=h1b[h][:, idx_lo + ro:idx_lo + ro + k, 1:65],
                        in_=ps[:, 0:k * W].rearrange("p (a b) -> p a b", a=k),
                        func=AF.Relu, bias=cs_t["b1s"][:, h:h + 1], scale=1.0)
            return h1b

        def conv2_band(c, h1b):
            h2b = [h2p.tile([128, CS], F32R, name=f"h2b{h}", tag=f"h2b{h}")
                   for h in range(2)]
            for h in range(2):
                ps = pcv.tile([128, CS], F32, name="cvps", tag="cvps")
                for t, (dy, dx) in enumerate(TAPS):
                    nc.tensor.matmul(
                        ps[:],
                        cs_t["w2L"][:, h, t, :],
                        h1b[h][:, 1 + dy:9 + dy, 1 + dx:65 + dx],
                        start=(t == 0), stop=(t == len(TAPS) - 1))
                nc.scalar.activation(
                    out=h2b[h][:], in_=ps[:],
                    func=AF.Relu, bias=cs_t["b2s"][:, h:h + 1], scale=1.0)
            return h2b

        def conv3_upt(sl, h2b):
            """u_pt[th] = w3P[:,th,:]^T @ h2b[h] + b3P (bias folded at the
            PSUM->SBUF copy). Relu'd conv2 output in, j-layout bf16 out."""
            u_pt = []
            for th in range(8):
                ps = pcv.tile([128, CS], F32, name="cvps", tag="cvps")
                nc.tensor.matmul(ps[:], cs_t["w3P"][:, th, :],
                                 h2b[th & 1][:], start=True, stop=True)
                u_t = upp.tile([128, CS], BF16, name=f"u{th}",
                               tag=f"u{sl}_{th}")
                nc.scalar.activation(out=u_t[:], in_=ps[:], func=AF.Identity,
                                     bias=cs_t["b3P"][:, th:th + 1], scale=1.0)
                u_pt.append(u_t)
            return u_pt

        # ------------------------------------------------ routing pieces
        def g_chain(n_sb, gpool, gtag):
            """g = n / ((0.5+n) * sqrt(n+1e-6)) -> bf16 [128, CS].
            ACT Sqrt (stage-batched to amortize table loads) + DVE STT/div."""
            rt = smt.tile([128, CS], F32, name="g_rt", tag="g_rt")
            nc.scalar.activation(out=rt[:], in_=n_sb[:], func=AF.Sqrt,
                                 bias=cb_eps[:], scale=1.0)
            den = smt.tile([128, CS], F32, name="g_den", tag="g_den")
            nc.vector.scalar_tensor_tensor(out=den[:], in0=n_sb[:], scalar=0.5,
                                           in1=rt[:], op0=OP.add, op1=OP.mult)
            rg = smt.tile([128, CS], F32, name="g_rg", tag="g_rg")
            nc.vector.reciprocal_approx_fast(out=rg[:], in_=den[:])
            g_t = gpool.tile([128, CS], BF16, name="g_g", tag=gtag)
            nc.vector.tensor_tensor(out=g_t[:], in0=n_sb[:], in1=rg[:],
                                    op=OP.mult)
            return g_t

        def accum_pass(u_pt, cT, masks, s_ps):
            """s_ps[q] = sum_(icg,h) cb*u_pt; cb[h] = quadrant shuffle of the
            scattered c tile."""
            cb = []
            for h in range(2):
                cbt = scr.tile([128, CS], BF16, name=f"cb{h}", tag=f"cb{h}")
                nc.vector.stream_shuffle(out=cbt[:], in_=cT[:],
                                         mask=MASK_CB[h])
                cb.append(cbt)
            for th in range(8):
                t, h = th >> 1, th & 1
                p_t = scr.tile([128, CS], BF16, name="pp", tag="pp")
                tt_eng().tensor_tensor(out=p_t[:], in0=u_pt[th][:],
                                       in1=cb[h][:], op=OP.mult)
                nc.tensor.matmul(s_ps[:], masks[:, t, :], p_t[:],
                                 start=(th == 0), stop=(th == 7))

        def d_pass(u_pt, s16, red_ps):
            """red_ps[16oc+ic] = sum_od u_pt*srep; drep[t] = quadrant
            shuffle of s16 (q-natural)."""
            drep = []
            for t in range(4):
                dt_ = scr.tile([128, CS], BF16, name=f"dr{t}", tag=f"dr{t}")
                nc.vector.stream_shuffle(out=dt_[:], in_=s16[:],
                                         mask=MASK_DREP[t])
                drep.append(dt_)
            for th in range(8):
                t = th >> 1
                q_t = scr.tile([128, CS], BF16, name="qq", tag="qq")
                tt_eng().tensor_tensor(out=q_t[:], in0=u_pt[th][:],
                                       in1=drep[t][:], op=OP.mult)
                nc.tensor.matmul(red_ps[:], cs_t["redD"][:, th, :], q_t[:],
                                 start=(th == 0), stop=(th == 7))

        # ------------------------------------------------ chunk state
        st = [dict() for _ in range(G)]

        def s_nsq(c, sl):
            u_pt = st[sl]["u"]
            nsq_ps = pred.tile([128, CS], F32, name="red", tag="red")
            for th in range(8):
                sq_t = scr.tile([128, CS], BF16, name="sq", tag="sq")
                tt_eng().tensor_tensor(out=sq_t[:], in0=u_pt[th][:],
                                       in1=u_pt[th][:], op=OP.mult)
                nc.tensor.matmul(nsq_ps[:], cs_t["redD"][:, th, :], sq_t[:],
                                 start=(th == 0), stop=(th == 7))
            n_sb = smt.tile([128, CS], F32, name="nsq", tag="nsq")
            nc.scalar.copy(out=n_sb[:], in_=nsq_ps[:])
            st[sl]["g_u"] = g_chain(n_sb, sm, f"gu_{sl}")

        def s_iter(c, sl, it):
            """One routing iteration: accum -> s16 -> ns -> g -> d -> b/c."""
            u_pt = st[sl]["u"]
            g_u = st[sl]["g_u"]
            if it == 1:
                cT, masks = g_u, cs_t["accMh"]
            else:
                cT, masks = st[sl]["ct2"], cs_t["accM"]
            s_ps = psa.tile([128, CS], F32, name="sacc", tag="sacc")
            accum_pass(u_pt, cT, masks, s_ps)
            s16 = sm.tile([128, CS], BF16, name="s16", tag=f"s16_{sl}")
            nc.scalar.copy(out=s16[:], in_=s_ps[:])
            # squash factor of s
            ssq = scr.tile([128, CS], BF16, name="ssq", tag="ssq")
            nc.scalar.activation(out=ssq[:], in_=s16[:], func=AF.Square)
            ns_ps = pred.tile([128, CS], F32, name="red", tag="red")
            nc.tensor.matmul(ns_ps[:], cs_t["onesB"][:], ssq[:],
                             start=True, stop=True)
            nsb = smt.tile([128, CS], F32, name="nsb", tag="nsb")
            nc.scalar.copy(out=nsb[:], in_=ns_ps[:])
            g_i = g_chain(nsb, smt, "g_i")
            # d = sum_od u*s ; b += d*g_u*g_i
            d_ps = pred.tile([128, CS], F32, name="red", tag="red")
            d_pass(u_pt, s16, d_ps)
            gg = smt.tile([128, CS], BF16, name="gg", tag="gg")
            nc.vector.tensor_tensor(out=gg[:], in0=g_i[:], in1=g_u[:],
                                    op=OP.mult)
            if it == 1:
                b2 = sm.tile([128, CS], F32, name="b2", tag=f"b2_{sl}")
                nc.vector.tensor_tensor(out=b2[:], in0=d_ps[:], in1=gg[:],
                                        op=OP.mult)
                st[sl]["b2"] = b2
                c2 = smt.tile([128, CS], BF16, name="c2", tag="c2")
                nc.scalar.activation(out=c2[:], in_=b2[:], func=AF.Sigmoid)
                ct2 = sm.tile([128, CS], BF16, name="ct2", tag=f"ct2_{sl}")
                nc.vector.tensor_tensor(out=ct2[:], in0=c2[:], in1=g_u[:],
                                        op=OP.mult)
                st[sl]["ct2"] = ct2
            else:
                tb = smt.tile([128, CS], F32, name="tb", tag="tb")
                nc.vector.tensor_tensor(out=tb[:], in0=d_ps[:], in1=gg[:],
                                        op=OP.mult)
                b3 = smt.tile([128, CS], F32, name="b3", tag="b3")
                nc.vector.tensor_tensor(out=b3[:], in0=tb[:],
                                        in1=st[sl]["b2"][:], op=OP.add)
                c3 = sm.tile([128, CS], BF16, name="c3", tag=f"c3_{sl}")
                nc.scalar.activation(out=c3[:], in_=b3[:], func=AF.Sigmoid)
                st[sl]["c3"] = c3

        def s_final(c, sl):
            csl = slice(c * CS, (c + 1) * CS)
            sf_ps = psa.tile([128, CS], F32, name="sacc", tag="sacc")
            accum_pass(st[sl]["u"], st[sl]["c3"], cs_t["accM"], sf_ps)
            nc.scalar.copy(out=sf_sb[:, csl], in_=sf_ps[:])

        # ------------------------------------------------ main loop
        for si in range(NCH // G):
            cs = list(range(si * G, (si + 1) * G))
            # conv1 one chunk ahead of conv2 so PE never waits on the ACT
            # relu copies of the same chunk
            h1b_cur = {cs[0]: conv1_band(cs[0])}
            h2bs = {}
            for k, c in enumerate(cs):
                if k + 1 < G:
                    h1b_cur[cs[k + 1]] = conv1_band(cs[k + 1])
                h2bs[c] = conv2_band(c, h1b_cur.pop(c))
            for c in cs:
                sl = c % G
                st[sl]["u"] = conv3_upt(sl, h2bs[c])
            for c in cs:
                s_nsq(c, c % G)
            for c in cs:
                s_iter(c, c % G, 1)
            for c in cs:
                s_iter(c, c % G, 2)
            for c in cs:
                s_final(c, c % G)

        if stage <= 4:
            ph2ps.close()
            nc.sync.dma_start(out=out_dram, in_=sf_sb[:])
            return

        # ---------------- tail: spatial capsule attention ----------------
        ph2ps.close()
        tailp = ctx.enter_context(tc.tile_pool(name="tailp", bufs=2))
        tt = ctx.enter_context(tc.tile_pool(name="tt", bufs=1))
        dramp = ctx.enter_context(tc.tile_pool(name="dramp", bufs=1,
                                               space="DRAM"))
        ppt = ctx.enter_context(tc.tile_pool(name="ppt", bufs=2, space="PSUM"))

        mh = tt.tile([128, 1], F32, name="mh", tag="mh")
        nc.vector.reduce_sum(out=mh[:], in_=sf_sb[:], axis=mybir.AxisListType.X)
        nc.scalar.mul(mh[:], mh[:], 1.0 / PX)

        # avg packed [64, CS]: partition 8c+oc holds chunk c's avg row oc
        for c in range(NCH):
            csl = slice(c * CS, (c + 1) * CS)
            scrc = tailp.tile([128, CS], F32R, name="p", tag="p")
            nc.vector.tensor_scalar(out=scrc[:], in0=sf_sb[:, csl],
                                    scalar1=mh[:], scalar2=None, op0=OP.mult)
            av_ps = ppt.tile([8, CS], F32, name="avgc", tag="avgc")
            nc.tensor.matmul(av_ps[:], cs_t["onesA"][:],
                             scrc[:], start=True, stop=True)
            # compute engines need 32-aligned start partitions; bounce via
            # SBUF and let DMA scatter to partition 8c
            avst = tailp.tile([8, CS], F32, name="avst", tag="avst")
            nc.scalar.copy(out=avst[:], in_=av_ps[:])
            nc.sync.dma_start(out=avg64[8 * c:8 * c + 8, :], in_=avst[:])

        rowsum = tt.tile([64, 1], F32, name="rowsum", tag="rowsum")
        nc.vector.reduce_sum(out=rowsum[:], in_=avg64[:],
                             axis=mybir.AxisListType.X)
        # gather the 64 per-(chunk,oc) row sums onto one partition, reduce
        # the chunk axis there, and broadcast back — avoids tiny PE matmuls
        rowsT = tt.tile([1, 64], F32, name="rowsT", tag="rowsT")
        nc.sync.dma_start(out=rowsT[:], in_=rowsum[:])
        m_row = tt.tile([1, 8], F32, name="m_row", tag="m_row")
        nc.vector.reduce_sum(
            out=m_row[:],
            in_=bass.AP(tensor=rowsT.tensor, offset=rowsT.offset,
                        ap=[[64, 1], [1, 8], [8, 8]]),
            axis=mybir.AxisListType.X)
        nc.scalar.mul(m_row[:], m_row[:], 1.0 / PX)
        mrow_d = dramp.tile([1, 8], F32, name="mrow_d", tag="mrow_d")
        nc.sync.dma_start(out=mrow_d[:], in_=m_row[:])
        m64 = tt.tile([64, 1], F32, name="m64", tag="m64")
        nc.sync.dma_start(
            out=m64[:],
            in_=bass.AP(tensor=mrow_d.tensor, offset=mrow_d.offset,
                        ap=[[0, 8], [1, 8]]))
        cen = tt.tile([64, CS], F32, name="cen", tag="cen")
        nc.vector.tensor_scalar(out=cen[:], in0=avg64[:], scalar1=m64[:],
                                scalar2=None, op0=OP.subtract)
        vjunk = tt.tile([64, CS], F32, name="vjunk", tag="vjunk")
        nc.vector.tensor_tensor(out=vjunk[:], in0=cen[:], in1=cen[:],
                                op=OP.mult)
        v64 = tt.tile([64, 1], F32, name="v64", tag="v64")
        nc.vector.reduce_sum(out=v64[:], in_=vjunk[:],
                             axis=mybir.AxisListType.X)
        vT = tt.tile([1, 64], F32, name="vT", tag="vT")
        nc.sync.dma_start(out=vT[:], in_=v64[:])
        var8 = tt.tile([1, 8], F32, name="var8", tag="var8")
        nc.vector.reduce_sum(
            out=var8[:],
            in_=bass.AP(tensor=vT.tensor, offset=vT.offset,
                        ap=[[64, 1], [1, 8], [8, 8]]),
            axis=mybir.AxisListType.X)
        cb_eps = tt.tile([1, 1], F32, name="cb_eps", tag="cb_eps")
        nc.vector.memset(cb_eps[:], 1e-6)
        sd8 = tt.tile([1, 8], F32, name="sd8", tag="sd8")
        nc.scalar.activation(out=sd8[:], in_=var8[:], func=AF.Sqrt,
                             bias=0.0, scale=1.0 / (PX - 1))
        nc.scalar.activation(out=sd8[:], in_=sd8[:], func=AF.Identity,
                             bias=cb_eps[:1], scale=1.0)
        rsd8 = tt.tile([1, 8], F32, name="rsd8", tag="rsd8")
        nc.vector.reciprocal(out=rsd8[:], in_=sd8[:])
        rsdw8 = tt.tile([1, 8], F32, name="rsdw8", tag="rsdw8")
        nc.vector.tensor_tensor(out=rsdw8[:], in0=rsd8[:], in1=cs_t["aw"][:],
                                op=OP.mult)
        rsdw_d = dramp.tile([1, 8], F32, name="rsdw_d", tag="rsdw_d")
        nc.sync.dma_start(out=rsdw_d[:], in_=rsdw8[:])
        rw64 = tt.tile([64, 1], F32, name="rw64", tag="rw64")
        nc.sync.dma_start(
            out=rw64[:],
            in_=bass.AP(tensor=rsdw_d.tensor, offset=rsdw_d.offset,
                        ap=[[0, 8], [1, 8]]))
        ab64 = tt.tile([64, 1], F32, name="ab64", tag="ab64")
        nc.sync.dma_start(
            out=ab64[:],
            in_=bass.AP(tensor=io["ab"].tensor, offset=io["ab"].offset,
                        ap=[[0, 8], [1, 8]]))
        t2 = tt.tile([64, CS], F32, name="t2", tag="t2")
        nc.vector.tensor_scalar(out=t2[:], in0=cen[:], scalar1=rw64[:],
                                scalar2=ab64[:], op0=OP.mult, op1=OP.add)
        sig = tt.tile([64, CS], BF16, name="sig", tag="sig")
        nc.scalar.activation(out=sig[:], in_=t2[:], func=AF.Sigmoid)

        for c in range(NCH):
            csl = slice(c * CS, (c + 1) * CS)
            srep = ppt.tile([128, CS], F32, name="srep", tag="srep")
            nc.tensor.matmul(srep[:], cs_t["selrep"][:, c, :],
                             sig[:], start=True, stop=True)
            o1 = tailp.tile([128, CS], F32, name="o1", tag="o1")
            nc.vector.tensor_tensor(out=o1[:], in0=srep[:], in1=sf_sb[:, csl],
                                    op=OP.mult)
            o2 = tailp.tile([128, CS], F32, name="o2", tag="o2")
            eng = nc.gpsimd if c % 2 == 0 else nc.vector
            eng.tensor_tensor(out=o2[:], in0=o1[:], in1=x_sb[:, csl],
                              op=OP.add)
            nc.sync.dma_start(out=out_dram[:, c * CS:(c + 1) * CS], in_=o2[:])


# ---------------------------------------------------------------- dispatch
_NC_CACHE = {}


def _get_nc():
    if "nc" not in _NC_CACHE:
        _NC_CACHE["nc"] = build_nc()
    return _NC_CACHE["nc"]


def kernel(x, w1, b1, w2, b2, w3, b3, attn_w, attn_b):
    x = np.ascontiguousarray(np.asarray(x, dtype=np.float32))
    consts = _prep_consts(
        np.asarray(w1, np.float32), np.asarray(b1, np.float32),
        np.asarray(w2, np.float32), np.asarray(b2, np.float32),
        np.asarray(w3, np.float32), np.asarray(b3, np.float32),
        np.asarray(attn_w, np.float32), np.asarray(attn_b, np.float32))
    consts = {k: np.ascontiguousarray(v) for k, v in consts.items()}

    nc = _get_nc()
    in_maps = []
    for b in range(B):
        m = {"x": x[b].reshape(128, PX).copy()}
        m.update(consts)
        in_maps.append(m)
    res = run_bass_kernel_spmd(nc, in_maps, core_ids=list(range(B)))
    out = np.zeros((B, 128, H, W), np.float32)
    for b in range(B):
        out[b] = res.results[b]["out"].reshape(128, H, W)
    return out
